# revision 36
# baseline (speedup 1.0000x reference)
"""MultiHeadAttention (GQA + symmetric ALiBi) on 8 trn2 NeuronCores.

Sharding: core = (batch n in {0,1}) x (head-group g in {0..3}).
Each core handles 4 query heads (one GQA pair of kv heads) for one batch.
All matmuls bf16 with fp32 PSUM accumulation; full-height 128-row
stationaries/moving operands throughout (sub-128 operands measured ~2x
slower: the PE HAM activity monitor keeps the clock gated at K=4/8).

Layout (vs v1 baseline at ~250us):
  - PE pre-warm: dummy matmuls during the initial DMA wait ramp the HAM
    clock gate so the K/V projections start at 2.4 GHz, not 1.2.
  - x^T resident in SBUF (8 tiles [128,2048]); phase A: K+V pass (one psum
    pool), then Q half 0, V transposes, Q half 1 (qps bufs=1 + tpps
    coexist in the 8 psum banks); drains split across DVE/ACT queues.
  - K^T for both kv heads stacked in one [128,S] tile (kt01); the S matmul
    uses the full [128,128] kt01 slice as stationary and the zero rows of
    the padded Q^T tiles select which kv head contributes.
  - Phase C loops (head, q-half, k-chunk): S psum [128,1024] double-
    buffered, pv accumulator [128,1024] (rows 65-127 garbage, never read),
    exp on ACT 1024-wide (the engine floor: 128 instrs x ~1.1us), texp
    multiplies on DVE (bf16 2x mode).
  - softmax: exp((S + alibi)/8) = exp(S*0.125) * texp, texp a host-built bf16
    table indexed by (i - u + 1920); no max-subtraction needed (args O(1)).
  - normalization: ones-column row sums -> DMA reshape to [128,8] ->
    reciprocal -> DRAM-bounce DMA broadcast to [64,1024] (0-stride source
    AP) -> DVE multiply. No PE involvement, so phase D's matmuls never
    queue behind the chain; drains/normalizes are deferred into the next
    half's chunk stream to avoid head-of-line blocking the DVE mul queue.
  - Phase D: partial = attnT @ Wo^T slice + bo/4; warm-keeper matmuls
    bridge the last normalize chain so the c=1 matmuls run at full clock;
    1024-wide bias adds split DVE/ACT; stores split across two DMA queues.
"""
import sys

sys.path.insert(0, "/opt/trn_rl_repo")
import numpy as np
import ml_dtypes

import concourse.bass as bass
import concourse.mybir as mybir
from concourse import bacc
from concourse.tile import TileContext
from concourse.masks import make_identity
from concourse.bass_utils import run_bass_kernel_spmd


def _register_ntff_hook_module():
    # bass_utils imports antenv.axon_hooks for trace=True under axon; this
    # image's antenv lacks it, so register an in-memory shim that wires the
    # NTFF profile hook straight to trn_agent_boot's ctypes implementation.
    import types

    if "antenv.axon_hooks" in sys.modules:
        return
    try:
        from trn_agent_boot.trn_boot import _ntff_profile_via_ctypes

        hook = _ntff_profile_via_ctypes("/opt/axon/libaxon_pjrt.so")
        mod = types.ModuleType("antenv.axon_hooks")
        mod._hook = hook
        mod.get_axon_ntff_profile_hook = lambda: mod._hook
        def _set(h):
            mod._hook = h
        mod.set_axon_ntff_profile_hook = _set
        sys.modules["antenv.axon_hooks"] = mod
    except Exception:
        pass


_register_ntff_hook_module()

S = 2048
E = 1024
D = 64
TW = 3968  # alibi exp-table width: u = j - k0 + 1920 in [0, 3968)
F32 = mybir.dt.float32
BF16 = mybir.dt.bfloat16

_NC = None
LAST_RESULTS = None


def _build():
    nc = bacc.Bacc("TRN2", target_bir_lowering=False, debug=False, num_devices=8)
    xT = nc.dram_tensor("xT", [E, S], BF16, kind="ExternalInput")
    wqT = nc.dram_tensor("wqT", [E, 256], BF16, kind="ExternalInput")
    wkT = nc.dram_tensor("wkT", [E, 128], BF16, kind="ExternalInput")
    wvT = nc.dram_tensor("wvT", [E, 128], BF16, kind="ExternalInput")
    woT = nc.dram_tensor("woT", [256, E], BF16, kind="ExternalInput")
    bo4 = nc.dram_tensor("bo4", [128, 8], F32, kind="ExternalInput")
    texp = nc.dram_tensor("texp", [4, 128, TW], BF16, kind="ExternalInput")
    outT = nc.dram_tensor("outT", [E, S], BF16, kind="ExternalOutput")
    # scratch for the reciprocal partition-broadcast bounce (slot per q-half)
    rdram = nc.dram_tensor("rdram", [8, 1024], BF16, kind="Internal")

    Exp = mybir.ActivationFunctionType.Exp

    with TileContext(nc) as tc:
        with (
            tc.sbuf_pool(name="const", bufs=1) as const,
            tc.sbuf_pool(name="qkv", bufs=1) as qkv,
            tc.sbuf_pool(name="pp", bufs=6) as pp,
            tc.sbuf_pool(name="norm", bufs=1) as norm,
        ):
            # ---- weights (emission order = DMA priority: KV pass needs
            # wk/wv + xt first; wq next; wo/bo/tex deferred below)
            wk_sb = const.tile([128, 8 * 128], BF16)
            nc.sync.dma_start(
                out=wk_sb.rearrange("p (c m) -> p c m", m=128),
                in_=wkT.rearrange("(c p) m -> p c m", p=128),
            )
            wv_sb = const.tile([128, 8 * 128], BF16)
            nc.sync.dma_start(
                out=wv_sb.rearrange("p (c m) -> p c m", m=128),
                in_=wvT.rearrange("(c p) m -> p c m", p=128),
            )
            # x^T resident: 8 tiles [128, 2048] (one per e-chunk)
            xt = [const.tile([128, S], BF16, name=f"xt{e}") for e in range(8)]
            for e in range(8):
                nc.sync.dma_start(out=xt[e], in_=xT[e * 128 : (e + 1) * 128, :])
            wq_sb = const.tile([128, 8 * 256], BF16)
            nc.sync.dma_start(
                out=wq_sb.rearrange("p (c m) -> p c m", m=256),
                in_=wqT.rearrange("(c p) m -> p c m", p=128),
            )
            wo_sb = const.tile([128, 2 * 1024], BF16)
            bo_sb = const.tile([128, 8], F32)
            tex_sb = const.tile([128, 4 * TW], BF16)
            ident = const.tile([128, 128], BF16)
            make_identity(nc, ident)
            # prefetch the Exp activation table during phase A (ACT_TABLE_LOAD
            # otherwise fires serially right before phase C's first exp)
            expwarm = const.tile([1, 1], F32)
            nc.scalar.activation(expwarm, ident[0:1, 0:1], Exp)

            # Q^T zero-padded to 128 rows (full-height stationaries/moving keep
            # the PE HAM activity monitor un-throttled). kv0 heads carry data in
            # rows 0:64, kv1 heads in rows 64:128; the S stationary is the full
            # [128,128] kt01 slice (both kv heads stacked) and the zero rows of
            # QT select which kv head contributes.
            QT = [qkv.tile([128, S], BF16, name=f"qt{h}") for h in range(4)]
            kt01 = qkv.tile([128, S], BF16, name="kt01")
            vt_sb = qkv.tile([128, S], BF16)
            # VS: per kv head, 16 chunk-slices of [128, 128]: 64 v dims + ones
            # column; cols 65-127 are uninitialized garbage feeding psum
            # partitions 65-127, which no drain ever reads.
            VS = [qkv.tile([128, 16 * 128], BF16, name=f"vs{k}") for k in range(2)]
            AT = [qkv.tile([128, S], BF16, name=f"at{c}") for c in range(2)]
            for h in range(2):
                nc.vector.memset(QT[h][64:128, :], 0.0)
            for h in range(2, 4):
                nc.vector.memset(QT[h][0:64, :], 0.0)
            for kv in range(2):
                nc.gpsimd.memset(
                    VS[kv].rearrange("p (c m) -> p c m", m=128)[:, :, 64:65], 1.0
                )

            # ---- Phase A0: PE pre-warm. The HAM clock gate ramps on ~3.4us of
            # activity; running dummy matmuls during the initial DMA wait means
            # the K/V projections start at full clock instead of 1.2 GHz.
            with tc.psum_pool(name="warmps", bufs=1) as wp:
                wt = wp.tile([128, 128], F32, tag="w", name="wt")
                for _ in range(40):
                    nc.tensor.matmul(
                        wt, ident, ident,
                        start=True, stop=True, skip_group_check=True,
                    )

            # ---- Phase A1: K and V projections (all e-chunks, full S)
            with tc.psum_pool(name="kvps", bufs=1) as kvp:
                ps_k = kvp.tile([128, S], F32, tag="psk", name="ps_k")
                ps_v = kvp.tile([128, S], F32, tag="psv", name="ps_v")
                for e in range(8):
                    st, sp = (e == 0), (e == 7)
                    for qq in range(4):
                        osl = slice(qq * 512, (qq + 1) * 512)
                        nc.tensor.matmul(
                            ps_k[:, osl], wk_sb[:, e * 128 : (e + 1) * 128],
                            xt[e][:, osl], start=st, stop=sp,
                        )
                        nc.tensor.matmul(
                            ps_v[:, osl], wv_sb[:, e * 128 : (e + 1) * 128],
                            xt[e][:, osl], start=st, stop=sp,
                        )
                # drains: DVE reads ps_k banks, ACT reads ps_v banks
                nc.vector.tensor_copy(kt01, ps_k)
                nc.scalar.copy(vt_sb, ps_v)

            # late-need constants: emitted here so their DMAs don't delay xt
            nc.sync.dma_start(
                out=wo_sb.rearrange("p (c m) -> p c m", m=1024),
                in_=woT.rearrange("(c p) m -> p c m", p=128),
            )
            nc.sync.dma_start(out=bo_sb, in_=bo4[:, :])
            for h in range(4):
                nc.sync.dma_start(out=tex_sb[:, h * TW : (h + 1) * TW], in_=texp[h])

            # ---- Phase A2/A3 interleaved: Q half 0, V transposes, Q half 1.
            # qps bufs=1 (4 banks) + tpps (4 banks) coexist; transposes run on
            # PE between the two Q passes while QT/VS drains overlap.
            with (
                tc.psum_pool(name="qps", bufs=1) as qp,
                tc.psum_pool(name="tpps", bufs=4) as tp,
            ):
                def q_pass(qh):
                    ps_q = qp.tile([128, S], F32, tag="psq", name="ps_q")
                    for e in range(8):
                        st, sp = (e == 0), (e == 7)
                        w = wq_sb[:, e * 256 + qh * 128 : e * 256 + (qh + 1) * 128]
                        for qq in range(4):
                            osl = slice(qq * 512, (qq + 1) * 512)
                            nc.tensor.matmul(
                                ps_q[:, osl], w, xt[e][:, osl], start=st, stop=sp,
                            )
                    # split drains across DVE/ACT queues: a second [64,2048]
                    # copy on DVE would head-of-line-block phase C's first
                    # texp multiplies behind it
                    r0 = 0 if qh == 0 else 64
                    nc.vector.tensor_copy(
                        QT[2 * qh][r0 : r0 + 64, :], ps_q[0:64, :]
                    )
                    nc.scalar.copy(
                        QT[2 * qh + 1][r0 : r0 + 64, :], ps_q[64:128, :]
                    )

                q_pass(0)
                for kc in range(16):
                    pt = tp.tile([128, 128], BF16, tag="tp", name="pt")
                    nc.tensor.transpose(pt, vt_sb[:, kc * 128 : (kc + 1) * 128], ident)
                    nc.vector.tensor_copy(
                        VS[0][:, kc * 128 : kc * 128 + 64], pt[:, 0:64]
                    )
                    nc.scalar.copy(
                        VS[1][:, kc * 128 : kc * 128 + 64], pt[:, 64:128]
                    )
                q_pass(1)

            # ---- Phase C: attention per (head, q-half)
            with (
                tc.psum_pool(name="sps", bufs=2) as spp,
                tc.psum_pool(name="pvps", bufs=2) as pvp,
            ):
                drain_q = []  # halves awaiting the drain/reciprocal chain
                norm_q = []   # halves awaiting the final at-multiply

                def emit_drain():
                    # pv psum -> sbuf; reciprocal runs 128-wide via a DMA
                    # reshape; a 0-stride DRAM-bounce DMA broadcasts the
                    # reciprocals across 64 partitions (no PE involvement, so
                    # phase D's matmuls never queue behind this chain)
                    h, qh, pv = drain_q.pop(0)
                    pvs = norm.tile([65, 1024], F32, tag="pvs", name="pvs", bufs=2)
                    nc.vector.tensor_copy(pvs, pv[0:65, :])
                    r128 = norm.tile([128, 8], F32, tag="r128", name="r128", bufs=2)
                    nc.gpsimd.dma_start(out=r128, in_=pvs[64:65, :])
                    rr = norm.tile([128, 8], F32, tag="rr", name="rr", bufs=2)
                    nc.vector.reciprocal(rr, r128)
                    rrb = norm.tile([128, 8], BF16, tag="rrb", name="rrb", bufs=2)
                    nc.vector.tensor_copy(rrb, rr)
                    slot = h * 2 + qh
                    nc.gpsimd.dma_start(out=rdram[slot], in_=rrb)
                    rbs = norm.tile([64, 1024], BF16, tag="rbs", name="rbs", bufs=2)
                    rd_ap = rdram[slot : slot + 1, :]
                    rbs_src = bass.AP(
                        tensor=rd_ap.tensor,
                        offset=rd_ap.offset,
                        ap=[[0, 64], [1, 1024]],
                    )
                    nc.gpsimd.dma_start(out=rbs, in_=rbs_src)
                    norm_q.append((h, qh, pvs[0:64, :], rbs))

                def emit_normalize():
                    h, qh, pvs, rbs = norm_q.pop(0)
                    at = AT[h // 2]
                    r0 = 64 * (h % 2)
                    nc.vector.tensor_mul(
                        at[r0 : r0 + 64, qh * 1024 : (qh + 1) * 1024],
                        pvs, rbs,
                    )

                def half_attention(h, qh):
                    kv = h // 2
                    q0 = qh * 1024
                    pv = pvp.tile([128, 1024], F32, tag="pv", name="pv")
                    for kc in range(16):
                        ks = slice(kc * 128, (kc + 1) * 128)
                        ss = spp.tile([128, 1024], F32, tag="s", name="ss")
                        for qq in range(2):
                            nc.tensor.matmul(
                                ss[:, qq * 512 : (qq + 1) * 512],
                                kt01[:, ks],
                                QT[h][:, q0 + qq * 512 : q0 + (qq + 1) * 512],
                                start=True, stop=True,
                            )
                        pexp = pp.tile([128, 1024], BF16, tag="pexp", name="pexp")
                        nc.scalar.activation(pexp, ss, Exp, scale=0.125)
                        p = pp.tile([128, 1024], BF16, tag="p", name="p")
                        u0 = h * TW + 1920 - kc * 128 + q0
                        nc.vector.tensor_mul(p, pexp, tex_sb[:, u0 : u0 + 1024])
                        vsl = VS[kv][:, kc * 128 : (kc + 1) * 128]
                        for qq in range(2):
                            nc.tensor.matmul(
                                pv[:, qq * 512 : (qq + 1) * 512],
                                vsl,
                                p[:, qq * 512 : (qq + 1) * 512],
                                start=(kc == 0), stop=(kc == 15),
                                skip_group_check=True,
                            )
                        # defer the previous halves' drain/normalize into this
                        # half's chunk stream: emitted at the head of a half
                        # they would head-of-line-block the DVE mul queue
                        if kc == 1 and drain_q:
                            emit_drain()
                        if kc == 3 and len(norm_q) > 1:
                            # only normalize halves whose reciprocal chain was
                            # emitted a full half ago (the rbs DMA chain takes
                            # ~10us; a younger at-mul would block the DVE queue)
                            emit_normalize()
                    drain_q.append((h, qh, pv))

                for h in range(4):
                    for qh in range(2):
                        half_attention(h, qh)
                while drain_q:
                    emit_drain()
                while norm_q:
                    emit_normalize()

            # ---- Phase D: output projection (+ bias/4)
            with (
                tc.psum_pool(name="ops", bufs=3) as op,
                tc.psum_pool(name="warm2", bufs=1) as wp2,
                tc.sbuf_pool(name="osb", bufs=4) as osb,
            ):
                for ec in range(8):
                    os_ = [op.tile([128, 1024], F32, tag="o", name="o") for _ in range(2)]
                    for c in range(2):
                        w = wo_sb[:, c * 1024 + ec * 128 : c * 1024 + (ec + 1) * 128]
                        for half in range(2):
                            for qq in range(2):
                                qs = slice(half * 1024 + qq * 512, half * 1024 + (qq + 1) * 512)
                                nc.tensor.matmul(
                                    os_[half][:, qq * 512 : (qq + 1) * 512],
                                    w, AT[c][:, qs],
                                    start=(c == 0), stop=(c == 1),
                                    skip_group_check=True,
                                )
                        if ec == 0 and c == 0:
                            # keep the HAM clock gate open while the c=1
                            # matmuls wait for the last head's normalize chain
                            wt2 = wp2.tile([128, 512], F32, tag="w2", name="wt2")
                            for _ in range(28):
                                nc.tensor.matmul(
                                    wt2, ident, kt01[:, 0:512],
                                    start=True, stop=True, skip_group_check=True,
                                )
                    for half in range(2):
                        o_sb = osb.tile([128, 1024], BF16, tag="osb", name="o_sb")
                        if half == 0:
                            nc.vector.tensor_scalar_add(
                                o_sb, os_[half], bo_sb[:, ec : ec + 1]
                            )
                        else:
                            nc.scalar.add(o_sb, os_[half], bo_sb[:, ec : ec + 1])
                        deng = nc.sync if half == 0 else nc.gpsimd
                        deng.dma_start(
                            out=outT[ec * 128 : (ec + 1) * 128,
                                     half * 1024 : (half + 1) * 1024],
                            in_=o_sb,
                        )

    nc.compile()
    return nc


def _texp_tables():
    i = np.arange(128, dtype=np.float64).reshape(128, 1)
    u = np.arange(TW, dtype=np.float64).reshape(1, TW)
    dist = np.abs(i + 1920.0 - u)
    tabs = []
    for g in range(4):
        tg = np.empty([4, 128, TW], dtype=ml_dtypes.bfloat16)
        for hh in range(4):
            slope = 2.0 ** (-(4 * g + hh + 1))
            tg[hh] = np.exp(-slope * dist / 8.0).astype(ml_dtypes.bfloat16)
        tabs.append(tg)
    return tabs


def kernel(x, Wq, Wk, Wv, Wo, bo, _trace=False, _trace_kwargs=None):
    global _NC, LAST_RESULTS
    x = np.asarray(x, dtype=np.float32)
    Wq = np.asarray(Wq, dtype=np.float32)
    Wk = np.asarray(Wk, dtype=np.float32)
    Wv = np.asarray(Wv, dtype=np.float32)
    Wo = np.asarray(Wo, dtype=np.float32)
    bo = np.asarray(bo, dtype=np.float32)

    if _NC is None:
        _NC = _build()
    nc = _NC

    tabs = _texp_tables()
    bf = ml_dtypes.bfloat16
    bo4 = np.ascontiguousarray((bo * 0.25).reshape(8, 128).T).astype(np.float32)
    in_maps = []
    for core in range(8):
        n, g = core // 4, core % 4
        hs = slice(4 * g * D, (4 * g + 4) * D)
        kvs = slice(2 * g * D, (2 * g + 2) * D)
        in_maps.append(
            {
                "xT": np.ascontiguousarray(x[n].T).astype(bf),
                "wqT": np.ascontiguousarray(Wq[hs].T).astype(bf),
                "wkT": np.ascontiguousarray(Wk[kvs].T).astype(bf),
                "wvT": np.ascontiguousarray(Wv[kvs].T).astype(bf),
                "woT": np.ascontiguousarray(Wo[:, hs].T).astype(bf),
                "bo4": bo4,
                "texp": tabs[g],
            }
        )

    kw = {}
    if _trace:
        kw["trace"] = True
        kw.update(_trace_kwargs or {})
    res = run_bass_kernel_spmd(nc, in_maps, list(range(8)), **kw)
    LAST_RESULTS = res

    out = np.empty((2, S, E), dtype=np.float32)
    for n in range(2):
        acc = res.results[n * 4]["outT"].astype(np.float32)
        for g in range(1, 4):
            acc = acc + res.results[n * 4 + g]["outT"]
        out[n] = acc.T
    return out
